# revision 1
# baseline (speedup 1.0000x reference)
"""GNN mean-aggregation message passing on 8 Trainium2 NeuronCores.

out[t] = mean_{e: tgt[e]==t} x[src[e]]   (0 if no incoming edges)

Strategy (target-sharded, uniform SPMD schedule):
  - Targets are dealt to cores serpentine-by-degree so per-(core,group) edge
    counts are balanced; each core owns 12544 output slots (98 groups of 128).
  - Host packs x as bf16 hi|lo pairs -> [N_pad, 128] bf16 (256B rows). The
    hi+lo split recovers ~fp32 precision after the f32 PSUM accumulation.
  - Edges are routed to the owning core, ordered by (supergroup, src-chunk,
    target) and packed into 128-edge slots. Every slot is bound (at compile
    time, uniformly across cores) to a target group g; its edges may only
    reference groups {g, g+1}. Two matmuls per slot (one-hot selection
    matrices vs. iota 0..127 / 128..255) accumulate into per-group PSUM.
  - Sources are gathered straight from HBM with dma_gather (int16 indices,
    4 chunks of 25088 rows to fit the int16 range). The SWDGE descriptor
    emission (~8ns/idx) is the wall; the schedule minimizes total idx count
    and keeps the last supergroup tiny so the matmul tail is short.
  - Finalize per group: (psum_hi + psum_lo) * inv_count -> DMA out.
"""
import sys

sys.path.insert(0, "/opt/trn_rl_repo")

import numpy as np
import ml_dtypes

bf16 = ml_dtypes.bfloat16

# ---- problem constants (hardcoded per harness contract) ----
N, F, E = 100000, 64, 1600000
P = 128
NCORES = 8
TPC = 12544                  # target slots per core (= 98 * 128)
GPC = TPC // P               # 98 groups per core
NCHUNKS = 4
CHUNK = 25088                # source rows per chunk (< 32768 for int16 idx)
NPAD = NCHUNKS * CHUNK       # 100352 padded source rows
SG_SIZES = [26, 26, 26, 19, 1]   # supergroups; tiny last one -> short tail
ELEM = 2 * F                 # 128 bf16 per packed row = 256B
SENT = 384.0                 # sentinel row id for pad edges (no iota match)
SEL_PIECE = 16               # slots per one-hot compare instruction
GCALL = 56                   # slots per dma_gather call (ring-wrap above ~8K idx)


def _chunk_order(sched, s):
    return sorted(range(NCHUNKS), key=lambda c: (-len(sched[s][c]), c))


def _balance_targets(tgt):
    """Serpentine-by-degree target -> (core, rank) assignment.

    Returns perm_o2n[old_target] = new_target_id (core*TPC + rank)."""
    deg = np.bincount(tgt, minlength=N)
    order = np.argsort(-deg, kind="stable")
    perm = np.empty(N, np.int64)
    pos = np.arange(N)
    rows = pos // NCORES
    cols = pos % NCORES
    cores = np.where(rows % 2 == 0, cols, NCORES - 1 - cols)
    for k in range(NCORES):
        ids = order[cores == k]
        perm[ids] = k * TPC + np.arange(ids.size)
    return perm


def _host_prep(x, edge_idx):
    """Build per-core device arrays and the shared slot schedule."""
    x = np.asarray(x, np.float32)
    src = np.asarray(edge_idx[0], np.int64)
    tgt_orig = np.asarray(edge_idx[1], np.int64)

    perm = _balance_targets(tgt_orig)
    tgt = perm[tgt_orig]

    # packed hi|lo bf16 table
    hi = x.astype(bf16)
    lo = (x - hi.astype(np.float32)).astype(bf16)
    xp = np.zeros((NPAD, ELEM), bf16)
    xp[:N, :F] = hi
    xp[:N, F:] = lo

    inv_cnt = np.bincount(tgt, minlength=NCORES * TPC).astype(np.float32)
    inv_cnt = 1.0 / np.maximum(inv_cnt, 1.0)

    core = tgt // TPC
    chunk = src // CHUNK
    gl = (tgt // P) - core * GPC          # local group 0..97
    sg_bounds = np.cumsum([0] + SG_SIZES)
    sg = np.searchsorted(sg_bounds, gl, side="right") - 1
    order = np.lexsort((tgt, chunk, sg, core))

    nsg = len(SG_SIZES)
    bin_id = (core * nsg + sg) * NCHUNKS + chunk
    bin_sizes = np.bincount(bin_id, minlength=NCORES * nsg * NCHUNKS)
    bin_starts = np.zeros(NCORES * nsg * NCHUNKS + 1, np.int64)
    np.cumsum(bin_sizes, out=bin_starts[1:])

    gl_sorted = gl[order]
    tgt_sorted = tgt[order]
    src_sorted = src[order]
    chunk_sorted = chunk[order]

    # ---- build shared schedule: per (sg, c) the block label list ----
    sched = []          # sched[sg][c] = np.array of block labels h (bin-local)
    for s in range(nsg):
        gs = SG_SIZES[s]
        row = []
        for c in range(NCHUNKS):
            e_kh = np.zeros((NCORES, gs), np.int64)
            for k in range(NCORES):
                b = (k * nsg + s) * NCHUNKS + c
                seg = gl_sorted[bin_starts[b]:bin_starts[b + 1]] - sg_bounds[s]
                if seg.size:
                    e_kh[k] = np.bincount(seg, minlength=gs)
            labels = []
            r = e_kh[:, 0].astype(np.int64)
            for h in range(gs):
                s_h = int(np.ceil(r / P).max())
                labels.extend([h] * s_h)
                cap = s_h * P - r
                if h + 1 < gs:
                    r = np.maximum(0, e_kh[:, h + 1] - cap)
                else:
                    assert (cap >= 0).all()
            row.append(np.asarray(labels, np.int64))
        sched.append(row)

    tot_slots = sum(len(row[c]) for row in sched for c in range(NCHUNKS))
    tot = tot_slots * P

    # ---- per-core edge placement into the uniform slot stream ----
    src_local = np.zeros((NCORES, tot), np.int16)
    trow = np.full((NCORES, tot), SENT, np.float32)
    spill = np.zeros(tot_slots, bool)   # slot has any lane in group h+1
    for k in range(NCORES):
        base = 0
        for s in range(nsg):
            for c in _chunk_order(sched, s):
                labels = sched[s][c]
                b = (k * nsg + s) * NCHUNKS + c
                lo_i, hi_i = bin_starts[b], bin_starts[b + 1]
                garr = gl_sorted[lo_i:hi_i] - sg_bounds[s]
                p = 0
                for bi, h in enumerate(labels):
                    upper = np.searchsorted(garr, h + 1, side="right")
                    take = min(P, upper - p)
                    if take > 0:
                        sl = slice(lo_i + p, lo_i + p + take)
                        pos = base + bi * P
                        src_local[k, pos:pos + take] = (
                            src_sorted[sl] - chunk_sorted[sl] * CHUNK
                        ).astype(np.int16)
                        trow[k, pos:pos + take] = (
                            tgt_sorted[sl] % P + P * (garr[p:p + take] - h)
                        ).astype(np.float32)
                        if (garr[p:p + take] > h).any():
                            spill[base // P + bi] = True
                        p += take
                assert p == hi_i - lo_i, (
                    f"core {k} sg {s} c {c}: placed {p} of {hi_i - lo_i}"
                )
                base += len(labels) * P
        assert base == tot

    # device layouts
    idx_dev = [
        np.tile(src_local[k].reshape(tot // 16, 16).T, (8, 1)).copy()
        for k in range(NCORES)
    ]
    trow_dev = [
        src_arr.reshape(tot // P, P).T.astype(bf16).copy()
        for src_arr in trow
    ]
    invc_dev = [
        inv_cnt[k * TPC:(k + 1) * TPC].reshape(GPC, P).T.copy()
        for k in range(NCORES)
    ]
    iota_dev = np.tile(np.arange(2 * P, dtype=np.float32), (P, 1)).astype(bf16)
    return xp, idx_dev, trow_dev, invc_dev, iota_dev, sched, tot, spill, perm


def _build_program(sched, tot, spill):
    from concourse import bacc, mybir, tile

    nsg = len(SG_SIZES)
    max_w = max(len(sched[s][c]) for s in range(nsg) for c in range(NCHUNKS))
    gsg_max = max(SG_SIZES)

    nc = bacc.Bacc(None, target_bir_lowering=False, debug=False)
    t_x = nc.dram_tensor("xp", [NPAD, ELEM], mybir.dt.bfloat16, kind="ExternalInput")
    t_idx = nc.dram_tensor("idx", [P, tot // 16], mybir.dt.int16, kind="ExternalInput")
    t_trow = nc.dram_tensor("trow", [P, tot // P], mybir.dt.bfloat16, kind="ExternalInput")
    t_invc = nc.dram_tensor("invc", [P, GPC], mybir.dt.float32, kind="ExternalInput")
    t_iota = nc.dram_tensor("iota", [P, 2 * P], mybir.dt.bfloat16, kind="ExternalInput")
    t_out = nc.dram_tensor("out", [TPC, F], mybir.dt.float32, kind="ExternalOutput")
    out_view = t_out[:].rearrange("(g r) f -> r g f", r=P)

    with tile.TileContext(nc) as tc:
        with (
            tc.tile_pool(name="const", bufs=1) as cpool,
            tc.tile_pool(name="msgs", bufs=4) as mpool,
            tc.tile_pool(name="sel", bufs=3) as spool,
            tc.tile_pool(name="stage", bufs=2) as stpool,
            tc.tile_pool(name="psum", bufs=8, space="PSUM") as ppool,
        ):
            idx_t = cpool.tile([P, tot // 16], mybir.dt.int16)
            trow_t = cpool.tile([P, tot // P], mybir.dt.bfloat16)
            invc_t = cpool.tile([P, GPC], mybir.dt.float32)
            iota_b = cpool.tile([P, 2 * P], mybir.dt.bfloat16)

            # first gather call's idx first, then everything else (fewer DMA
            # instructions -> less Sync-sequencer serialization + sem aliasing)
            w_first = min(GCALL, len(sched[0][_chunk_order(sched, 0)[0]]))
            nc.sync.dma_start(out=iota_b[:], in_=t_iota[:])   # primes cold DMA queue
            nc.sync.dma_start(out=idx_t[:, :w_first * 8],
                              in_=t_idx[:, :w_first * 8])
            meta_loaded = [False]

            def _load_rest_meta():
                nc.sync.dma_start(out=idx_t[:, w_first * 8:],
                                  in_=t_idx[:, w_first * 8:])
                nc.sync.dma_start(out=trow_t[:], in_=t_trow[:])
                nc.sync.dma_start(out=invc_t[:], in_=t_invc[:])
                meta_loaded[0] = True

            slot_off = 0     # global slot offset in the stream
            g_base = 0       # global group offset
            for s in range(nsg):
                gs = SG_SIZES[s]
                nslots_psum = gs + 1
                nbanks = (nslots_psum + 3) // 4
                pts = [
                    ppool.tile([P, 4 * P], mybir.dt.float32, name=f"ps{s}_{b}", tag="ps")
                    for b in range(nbanks)
                ]
                for pt in pts:
                    nc.vector.memset(pt[:], 0.0)

                def pslot(j):
                    return pts[j // 4][:, (j % 4) * P:(j % 4 + 1) * P]

                for c in _chunk_order(sched, s):
                    labels = sched[s][c]
                    w = len(labels)
                    if w == 0:
                        continue
                    msgs_t = mpool.tile([P, w, ELEM], mybir.dt.bfloat16, name="msgs")
                    # split big bins into <=GCALL-slot gather calls: much above
                    # ~8K idx/call hits SWDGE ring-wrap stalls
                    for g0 in range(0, w, GCALL):
                        gw = min(GCALL, w - g0)
                        nc.gpsimd.dma_gather(
                            out_ap=msgs_t[:, g0:g0 + gw, :],
                            in_ap=t_x[c * CHUNK:(c + 1) * CHUNK, :],
                            idxs_ap=idx_t[:, (slot_off + g0) * 8:(slot_off + g0 + gw) * 8],
                            num_idxs=gw * P,
                            num_idxs_reg=gw * P,
                            elem_size=ELEM,
                            single_packet=False,
                        )
                        if not meta_loaded[0]:
                            _load_rest_meta()
                    sidx0 = slot_off
                    for p0 in range(0, w, SEL_PIECE):
                        pw = min(SEL_PIECE, w - p0)
                        sel_t = spool.tile([P, SEL_PIECE, 2 * P], mybir.dt.bfloat16, name="sel")
                        nc.vector.tensor_tensor(
                            out=sel_t[:, :pw, :],
                            in0=trow_t[:, slot_off + p0:slot_off + p0 + pw]
                            .to_broadcast([P, pw, 2 * P]),
                            in1=iota_b[:, None, :].to_broadcast([P, pw, 2 * P]),
                            op=mybir.AluOpType.is_equal,
                        )
                        for si in range(pw):
                            h = int(labels[p0 + si])
                            halves = ((0, h), (1, h + 1)) if spill[sidx0 + p0 + si] else ((0, h),)
                            for half, j in halves:
                                nc.tensor.matmul(
                                    pslot(j),
                                    lhsT=sel_t[:, si, half * P:(half + 1) * P],
                                    rhs=msgs_t[:, p0 + si, :],
                                    start=False,
                                    stop=False,
                                    skip_group_check=True,
                                )
                    slot_off += w

                stage_t = stpool.tile([P, gsg_max, F], mybir.dt.float32, name="stage")
                for j in range(gs):
                    tmp_t = stpool.tile([P, F], mybir.dt.float32, name="tmp", tag="tmp")
                    nc.vector.tensor_copy(out=tmp_t[:], in_=pslot(j)[:, 0:F])
                    nc.vector.tensor_add(
                        out=stage_t[:, j, :],
                        in0=tmp_t[:],
                        in1=pslot(j)[:, F:2 * F],
                    )
                    nc.vector.tensor_tensor(
                        out=stage_t[:, j, :],
                        in0=stage_t[:, j, :],
                        in1=invc_t[:, g_base + j, None].to_broadcast([P, F]),
                        op=mybir.AluOpType.mult,
                    )
                nc.sync.dma_start(
                    out=out_view[:, g_base:g_base + gs, :],
                    in_=stage_t[:, :gs, :],
                )
                g_base += gs

    nc.compile()
    return nc


def kernel(x, edge_idx):
    from concourse.bass_utils import run_bass_kernel_spmd

    xp, idx_dev, trow_dev, invc_dev, iota_dev, sched, tot, spill, perm = _host_prep(x, edge_idx)
    nc = _build_program(sched, tot, spill)
    in_maps = [
        {"xp": xp, "idx": idx_dev[k], "trow": trow_dev[k], "invc": invc_dev[k],
         "iota": iota_dev}
        for k in range(NCORES)
    ]
    res = run_bass_kernel_spmd(nc, in_maps, list(range(NCORES)))
    dev = np.concatenate([res.results[k]["out"] for k in range(NCORES)], axis=0)
    return dev[perm]



# revision 5
# speedup vs baseline: 2.3632x; 2.3632x over previous
"""GNN mean-aggregation message passing on 8 Trainium2 NeuronCores.

out[t] = mean_{e: tgt[e]==t} x[src[e]]   (0 if no incoming edges)

Strategy (target-sharded, uniform SPMD schedule):
  - Targets are dealt to cores serpentine-by-degree so per-(core,group) edge
    counts are balanced; each core owns 12544 output slots (98 groups of 128).
  - Host packs x as bf16 hi|lo pairs -> [N_pad, 128] bf16 (256B rows). The
    hi+lo split recovers ~fp32 precision after the f32 PSUM accumulation.
  - Edges are routed to the owning core, ordered by (supergroup, src-chunk,
    target) and packed into 128-edge slots. Every slot is bound (at compile
    time, uniformly across cores) to a target group g; its edges may only
    reference groups {g, g+1}. Two matmuls per slot (one-hot selection
    matrices vs. iota 0..127 / 128..255) accumulate into per-group PSUM.
  - Sources are gathered straight from HBM with dma_gather (int16 indices,
    4 chunks of 25088 rows to fit the int16 range). The SWDGE descriptor
    emission (~8ns/idx) is the wall; the schedule minimizes total idx count
    and keeps the last supergroup tiny so the matmul tail is short.
  - Finalize per group: (psum_hi + psum_lo) * inv_count -> DMA out.
"""
import sys

sys.path.insert(0, "/opt/trn_rl_repo")

import numpy as np
import ml_dtypes

bf16 = ml_dtypes.bfloat16

# ---- problem constants (hardcoded per harness contract) ----
N, F, E = 100000, 64, 1600000
P = 128
NCORES = 8
TPC = 12544                  # target slots per core (= 98 * 128)
GPC = TPC // P               # 98 groups per core
NCHUNKS = 4
CHUNK = 25088                # source rows per chunk (< 32768 for int16 idx)
NPAD = NCHUNKS * CHUNK       # 100352 padded source rows
SG_SIZES = [26, 26, 26, 19, 1]   # supergroups; tiny last one -> short tail
ELEM = 2 * F                 # 128 bf16 per packed row = 256B
SENT = 384.0                 # sentinel row id for pad edges (no iota match)
SEL_PIECE = 16               # slots per one-hot compare instruction
GCALL = 56                   # slots per dma_gather call (ring-wrap above ~8K idx)
NQUEUES = 4                  # SWDGE queues; queue q emits on Q7 core pair {2q,2q+1}


def _chunk_order(sched, s):
    return sorted(range(NCHUNKS), key=lambda c: (-len(sched[s][c]), c))


def _balance_targets(tgt):
    """Serpentine-by-degree target -> (core, rank) assignment.

    Returns perm_o2n[old_target] = new_target_id (core*TPC + rank)."""
    deg = np.bincount(tgt, minlength=N)
    order = np.argsort(-deg, kind="stable")
    perm = np.empty(N, np.int64)
    pos = np.arange(N)
    rows = pos // NCORES
    cols = pos % NCORES
    cores = np.where(rows % 2 == 0, cols, NCORES - 1 - cols)
    for k in range(NCORES):
        ids = order[cores == k]
        perm[ids] = k * TPC + np.arange(ids.size)
    return perm


def _host_prep(x, edge_idx):
    """Build per-core device arrays and the shared slot schedule."""
    x = np.asarray(x, np.float32)
    src = np.asarray(edge_idx[0], np.int64)
    tgt_orig = np.asarray(edge_idx[1], np.int64)

    perm = _balance_targets(tgt_orig)
    tgt = perm[tgt_orig]

    # packed hi|lo bf16 table
    hi = x.astype(bf16)
    lo = (x - hi.astype(np.float32)).astype(bf16)
    xp = np.zeros((NPAD, ELEM), bf16)
    xp[:N, :F] = hi
    xp[:N, F:] = lo

    inv_cnt = np.bincount(tgt, minlength=NCORES * TPC).astype(np.float32)
    inv_cnt = 1.0 / np.maximum(inv_cnt, 1.0)

    core = tgt // TPC
    chunk = src // CHUNK
    gl = (tgt // P) - core * GPC          # local group 0..97
    sg_bounds = np.cumsum([0] + SG_SIZES)
    sg = np.searchsorted(sg_bounds, gl, side="right") - 1
    order = np.lexsort((tgt, chunk, sg, core))

    nsg = len(SG_SIZES)
    bin_id = (core * nsg + sg) * NCHUNKS + chunk
    bin_sizes = np.bincount(bin_id, minlength=NCORES * nsg * NCHUNKS)
    bin_starts = np.zeros(NCORES * nsg * NCHUNKS + 1, np.int64)
    np.cumsum(bin_sizes, out=bin_starts[1:])

    gl_sorted = gl[order]
    tgt_sorted = tgt[order]
    src_sorted = src[order]
    chunk_sorted = chunk[order]

    # ---- build shared schedule: per (sg, c) the block label list ----
    sched = []          # sched[sg][c] = np.array of block labels h (bin-local)
    for s in range(nsg):
        gs = SG_SIZES[s]
        row = []
        for c in range(NCHUNKS):
            e_kh = np.zeros((NCORES, gs), np.int64)
            for k in range(NCORES):
                b = (k * nsg + s) * NCHUNKS + c
                seg = gl_sorted[bin_starts[b]:bin_starts[b + 1]] - sg_bounds[s]
                if seg.size:
                    e_kh[k] = np.bincount(seg, minlength=gs)
            labels = []
            r = e_kh[:, 0].astype(np.int64)
            for h in range(gs):
                s_h = int(np.ceil(r / P).max())
                labels.extend([h] * s_h)
                cap = s_h * P - r
                if h + 1 < gs:
                    r = np.maximum(0, e_kh[:, h + 1] - cap)
                else:
                    assert (cap >= 0).all()
            row.append(np.asarray(labels, np.int64))
        sched.append(row)

    tot_slots = sum(len(row[c]) for row in sched for c in range(NCHUNKS))
    tot = tot_slots * P

    # ---- per-core edge placement into the uniform slot stream ----
    src_local = np.zeros((NCORES, tot), np.int16)
    trow = np.full((NCORES, tot), SENT, np.float32)
    spill = np.zeros(tot_slots, bool)   # slot has any lane in group h+1
    for k in range(NCORES):
        base = 0
        for s in range(nsg):
            for c in _chunk_order(sched, s):
                labels = sched[s][c]
                b = (k * nsg + s) * NCHUNKS + c
                lo_i, hi_i = bin_starts[b], bin_starts[b + 1]
                garr = gl_sorted[lo_i:hi_i] - sg_bounds[s]
                p = 0
                for bi, h in enumerate(labels):
                    upper = np.searchsorted(garr, h + 1, side="right")
                    take = min(P, upper - p)
                    if take > 0:
                        sl = slice(lo_i + p, lo_i + p + take)
                        pos = base + bi * P
                        src_local[k, pos:pos + take] = (
                            src_sorted[sl] - chunk_sorted[sl] * CHUNK
                        ).astype(np.int16)
                        trow[k, pos:pos + take] = (
                            tgt_sorted[sl] % P + P * (garr[p:p + take] - h)
                        ).astype(np.float32)
                        if (garr[p:p + take] > h).any():
                            spill[base // P + bi] = True
                        p += take
                assert p == hi_i - lo_i, (
                    f"core {k} sg {s} c {c}: placed {p} of {hi_i - lo_i}"
                )
                base += len(labels) * P
        assert base == tot

    # device layouts
    idx_dev = [
        np.tile(src_local[k].reshape(tot // 16, 16).T, (8, 1)).copy()
        for k in range(NCORES)
    ]
    trow_dev = [
        src_arr.reshape(tot // P, P).T.astype(bf16).copy()
        for src_arr in trow
    ]
    invc_dev = [
        inv_cnt[k * TPC:(k + 1) * TPC].reshape(GPC, P).T.copy()
        for k in range(NCORES)
    ]
    iota_dev = np.tile(np.arange(2 * P, dtype=np.float32), (P, 1)).astype(bf16)
    return xp, idx_dev, trow_dev, invc_dev, iota_dev, sched, tot, spill, perm


def _build_program(sched, tot, spill):
    from concourse import bacc, mybir, tile

    nsg = len(SG_SIZES)
    max_w = max(len(sched[s][c]) for s in range(nsg) for c in range(NCHUNKS))
    gsg_max = max(SG_SIZES)

    nc = bacc.Bacc(None, target_bir_lowering=False, debug=False,
                   num_swdge_queues=NQUEUES)
    t_x = nc.dram_tensor("xp", [NPAD, ELEM], mybir.dt.bfloat16, kind="ExternalInput")
    t_idx = nc.dram_tensor("idx", [P, tot // 16], mybir.dt.int16, kind="ExternalInput")
    t_trow = nc.dram_tensor("trow", [P, tot // P], mybir.dt.bfloat16, kind="ExternalInput")
    t_invc = nc.dram_tensor("invc", [P, GPC], mybir.dt.float32, kind="ExternalInput")
    t_iota = nc.dram_tensor("iota", [P, 2 * P], mybir.dt.bfloat16, kind="ExternalInput")
    t_out = nc.dram_tensor("out", [TPC, F], mybir.dt.float32, kind="ExternalOutput")
    out_view = t_out[:].rearrange("(g r) f -> r g f", r=P)

    with tile.TileContext(nc) as tc:
        with (
            tc.tile_pool(name="const", bufs=1) as cpool,
            tc.tile_pool(name="msgs", bufs=4) as mpool,
            tc.tile_pool(name="sel", bufs=3) as spool,
            tc.tile_pool(name="stage", bufs=2) as stpool,
            tc.tile_pool(name="psum", bufs=8, space="PSUM") as ppool,
        ):
            idx_t = cpool.tile([P, tot // 16], mybir.dt.int16)
            trow_t = cpool.tile([P, tot // P], mybir.dt.bfloat16)
            invc_t = cpool.tile([P, GPC], mybir.dt.float32)
            iota_b = cpool.tile([P, 2 * P], mybir.dt.bfloat16)

            # first gather call's idx first, then everything else (fewer DMA
            # instructions -> less Sync-sequencer serialization + sem aliasing)
            w_first = min(GCALL, len(sched[0][_chunk_order(sched, 0)[0]]))
            nc.sync.dma_start(out=iota_b[:], in_=t_iota[:])   # primes cold DMA queue
            nc.sync.dma_start(out=idx_t[:, :w_first * 8],
                              in_=t_idx[:, :w_first * 8])
            meta_loaded = [False]
            qctr = [0]

            def _load_rest_meta():
                nc.sync.dma_start(out=idx_t[:, w_first * 8:],
                                  in_=t_idx[:, w_first * 8:])
                nc.sync.dma_start(out=trow_t[:], in_=t_trow[:])
                nc.sync.dma_start(out=invc_t[:], in_=t_invc[:])
                meta_loaded[0] = True

            slot_off = 0     # global slot offset in the stream
            g_base = 0       # global group offset
            for s in range(nsg):
                gs = SG_SIZES[s]
                nslots_psum = gs + 1
                nbanks = (nslots_psum + 3) // 4
                pts = [
                    ppool.tile([P, 4 * P], mybir.dt.float32, name=f"ps{s}_{b}", tag="ps")
                    for b in range(nbanks)
                ]
                for pt in pts:
                    nc.vector.memset(pt[:], 0.0)

                def pslot(j):
                    return pts[j // 4][:, (j % 4) * P:(j % 4 + 1) * P]

                for c in _chunk_order(sched, s):
                    labels = sched[s][c]
                    w = len(labels)
                    if w == 0:
                        continue
                    msgs_t = mpool.tile([P, w, ELEM], mybir.dt.bfloat16, name="msgs")
                    # split big bins into <=GCALL-slot gather calls: much above
                    # ~8K idx/call hits SWDGE ring-wrap stalls. Round-robin the
                    # 4 SWDGE queues so descriptor emission runs on all 4 Q7
                    # core pairs concurrently.
                    for g0 in range(0, w, GCALL):
                        gw = min(GCALL, w - g0)
                        nc.gpsimd.dma_gather(
                            out_ap=msgs_t[:, g0:g0 + gw, :],
                            in_ap=t_x[c * CHUNK:(c + 1) * CHUNK, :],
                            idxs_ap=idx_t[:, (slot_off + g0) * 8:(slot_off + g0 + gw) * 8],
                            num_idxs=gw * P,
                            num_idxs_reg=gw * P,
                            elem_size=ELEM,
                            single_packet=False,
                            queue_num=qctr[0] % NQUEUES,
                        )
                        qctr[0] += 1
                        if not meta_loaded[0]:
                            _load_rest_meta()
                    sidx0 = slot_off
                    for p0 in range(0, w, SEL_PIECE):
                        pw = min(SEL_PIECE, w - p0)
                        sel_t = spool.tile([P, SEL_PIECE, 2 * P], mybir.dt.bfloat16, name="sel")
                        nc.vector.tensor_tensor(
                            out=sel_t[:, :pw, :],
                            in0=trow_t[:, slot_off + p0:slot_off + p0 + pw]
                            .to_broadcast([P, pw, 2 * P]),
                            in1=iota_b[:, None, :].to_broadcast([P, pw, 2 * P]),
                            op=mybir.AluOpType.is_equal,
                        )
                        for si in range(pw):
                            h = int(labels[p0 + si])
                            halves = ((0, h), (1, h + 1)) if spill[sidx0 + p0 + si] else ((0, h),)
                            for half, j in halves:
                                nc.tensor.matmul(
                                    pslot(j),
                                    lhsT=sel_t[:, si, half * P:(half + 1) * P],
                                    rhs=msgs_t[:, p0 + si, :],
                                    start=False,
                                    stop=False,
                                    skip_group_check=True,
                                )
                    slot_off += w

                stage_t = stpool.tile([P, gsg_max, F], mybir.dt.float32, name="stage")
                for j in range(gs):
                    tmp_t = stpool.tile([P, F], mybir.dt.float32, name="tmp", tag="tmp")
                    nc.vector.tensor_copy(out=tmp_t[:], in_=pslot(j)[:, 0:F])
                    nc.vector.tensor_add(
                        out=stage_t[:, j, :],
                        in0=tmp_t[:],
                        in1=pslot(j)[:, F:2 * F],
                    )
                    nc.vector.tensor_tensor(
                        out=stage_t[:, j, :],
                        in0=stage_t[:, j, :],
                        in1=invc_t[:, g_base + j, None].to_broadcast([P, F]),
                        op=mybir.AluOpType.mult,
                    )
                nc.sync.dma_start(
                    out=out_view[:, g_base:g_base + gs, :],
                    in_=stage_t[:, :gs, :],
                )
                g_base += gs

    nc.compile()
    return nc


def kernel(x, edge_idx):
    from concourse.bass_utils import run_bass_kernel_spmd

    xp, idx_dev, trow_dev, invc_dev, iota_dev, sched, tot, spill, perm = _host_prep(x, edge_idx)
    nc = _build_program(sched, tot, spill)
    in_maps = [
        {"xp": xp, "idx": idx_dev[k], "trow": trow_dev[k], "invc": invc_dev[k],
         "iota": iota_dev}
        for k in range(NCORES)
    ]
    res = run_bass_kernel_spmd(nc, in_maps, list(range(NCORES)))
    dev = np.concatenate([res.results[k]["out"] for k in range(NCORES)], axis=0)
    return dev[perm]

